# revision 1
# baseline (speedup 1.0000x reference)
"""Multi-head attention (b=4, n=2048, dm=1024, h=16) on 8 TRN2 NeuronCores.

Sharding: batch (4) x head-group (2) -> 8 cores, Megatron-style.
Core c handles batch c//2 and heads [8*(c%2), 8*(c%2)+8). Each core:
  1. QKV projection for its 8 heads over its batch's 2048 tokens
     (q,k produced transposed [d, tok]; v produced natural [tok, d]).
  2. Attention per head-pair (heads packed at partitions 0:64 / 64:128 so
     the K=64 score matmuls overlap in the PE array via row groups).
     Scores computed as S^T = k^T q (keys on partitions) so softmax needs
     no transposes; exp on ScalarE with the 1/sqrt(dm) scale folded in;
     ctx^T = v'^T E with a ones column in v' producing the softmax
     denominator for free (M=65).  Softmax division: denominator row ->
     K=1 ones-matmul broadcast (PE) -> reciprocal_approx_fast -> multiply
     (VectorE). Odd heads are moved to partitions 64:128 with an
     identity matmul.
  3. Partial output projection ctx @ w_out[:, cols].T -> [2048, 1024].
Host sums the two partials per batch and adds the bias.

Matmuls run in bf16 (inputs cast on device); softmax denominator path in
fp32r.
"""

import numpy as np

import concourse.bass as bass
import concourse.tile as tile
from concourse import bacc, mybir
from concourse import bass_utils
from concourse.masks import make_identity

f32 = mybir.dt.float32
f32r = mybir.dt.float32r
bf16 = mybir.dt.bfloat16
f8 = mybir.dt.float8e4
Exp = mybir.ActivationFunctionType.Exp
Mult = mybir.AluOpType.mult

TOK = 2048          # tokens per batch
DM = 1024           # model dim
DL = 512            # local q/k/v feature dim (8 heads x 64)
D = 64              # head dim
NH = 8              # local heads
NPAIR = 4           # head pairs (partition blocks of ctx/qk)
KT = 8              # dm / 128 contraction tiles
SCALE = DM ** (-0.5)
N_CORES = 8


def _build(tc, xT, wqkT, wvT, woutT, out_p):
    nc = tc.nc

    ctp = tc.alloc_tile_pool(name="ctp", bufs=1)
    qkp = tc.alloc_tile_pool(name="qkp", bufs=1)
    vp_ = tc.alloc_tile_pool(name="vp", bufs=1)

    ctx_sb = ctp.tile([128, NPAIR, TOK], bf16, tag="ctx")    # 16 KB/part
    qk_sb = qkp.tile([128, 2 * NPAIR, TOK], bf16, tag="qk")  # 32 KB/part
    v_sb = vp_.tile([128, 16, NH, D + 1], bf16, tag="v")     # 17 KB/part

    # ones column of v' (softmax denominator accumulator)
    nc.vector.memset(v_sb[:, :, :, D:D + 1], 1.0)

    # ---- stage 1: QKV projection, token-quarters of 512 ----
    with (
        tc.tile_pool(name="w1", bufs=1) as wp,
        tc.tile_pool(name="ws", bufs=2) as wsp,
        tc.tile_pool(name="x", bufs=10) as xp,
        tc.tile_pool(name="xs", bufs=4) as xsp,
        tc.tile_pool(name="psq", bufs=4, space="PSUM") as psq,
    ):
        wqk_sb = wp.tile([128, KT, 2 * DL], bf16, tag="wqk")   # 16 KB/part
        wv_sb = wp.tile([128, KT, DL], bf16, tag="wv")         # 8 KB/part
        for kt in range(KT):
            wst = wsp.tile([128, 2 * DL], f32, tag="wst")
            nc.sync.dma_start(wst[:], wqkT[kt * 128:(kt + 1) * 128, :])
            with nc.allow_low_precision(reason="bf16 cast"):
                nc.vector.tensor_copy(wqk_sb[:, kt, :], wst[:])
            wst2 = wsp.tile([128, DL], f32, tag="wst2")
            nc.sync.dma_start(wst2[:], wvT[kt * 128:(kt + 1) * 128, :])
            with nc.allow_low_precision(reason="bf16 cast"):
                nc.vector.tensor_copy(wv_sb[:, kt, :], wst2[:])
        for tq in range(4):
            ts512 = slice(tq * 512, (tq + 1) * 512)
            xq = []
            for kt in range(KT):
                xst = xsp.tile([128, 512], f32, tag="xst")
                nc.sync.dma_start(xst[:], xT[kt * 128:(kt + 1) * 128, ts512])
                xk = xp.tile([128, 512], bf16, tag="x")
                with nc.allow_low_precision(reason="bf16 cast"):
                    nc.vector.tensor_copy(xk[:], xst[:])
                xq.append(xk)
            # q/k (transposed layout): out [feat 128, tok 512]
            for f in range(8):
                ps = psq.tile([128, 512], f32, tag="qk")
                for kt in range(KT):
                    nc.tensor.matmul(
                        ps[:], wqk_sb[:, kt, f * 128:(f + 1) * 128],
                        xq[kt][:], start=(kt == 0), stop=(kt == KT - 1))
                with nc.allow_low_precision(reason="bf16"):
                    nc.vector.tensor_copy(qk_sb[:, f, ts512], ps[:])
            # v (natural layout): out [tok 128, feat 512]
            for tt in range(4):
                jt = tq * 4 + tt
                ps = psq.tile([128, 512], f32, tag="v")
                for kt in range(KT):
                    nc.tensor.matmul(
                        ps[:], xq[kt][:, tt * 128:(tt + 1) * 128],
                        wv_sb[:, kt, :], start=(kt == 0), stop=(kt == KT - 1))
                with nc.allow_low_precision(reason="bf16 v"):
                    nc.vector.tensor_copy(
                        v_sb[:, jt, :, 0:D],
                        ps[:].rearrange("p (h d) -> p h d", h=NH))

    # ---- stage 2: attention, per head-pair, i-halves of 1024.
    # Heads 2p / 2p+1 live at partitions 0:64 / 64:128 and their K=64
    # score matmuls overlap in the PE array via row groups.  Full-array
    # heater matmuls keep the PE activity monitor warm (partial-array
    # matmuls alone leave the clock gated at half rate).
    with (
        tc.tile_pool(name="psS", bufs=1, space="PSUM") as psS,
        tc.tile_pool(name="psC", bufs=1, space="PSUM") as psC,
        tc.tile_pool(name="ep", bufs=4) as ep,
        tc.tile_pool(name="dv", bufs=2) as dv,
        tc.tile_pool(name="on", bufs=1) as onp,
    ):
        # ones row at partition 64 for the K=1 denominator-broadcast matmul
        ones_t = onp.tile([65, D], f32r, tag="ones")
        nc.vector.memset(ones_t[:].bitcast(f32), 1.0)
        # identity for the odd-head partition shift
        ident = onp.tile([64, 64], bf16, tag="ident")
        make_identity(nc, ident[:])
        for p in range(NPAIR):
            for ih in range(2):
                ihs = slice(ih * 1024, (ih + 1) * 1024)
                ps_ca = psC.tile([65, 1024], f32, tag="CA")
                ps_cb = psC.tile([65, 1024], f32, tag="CB")
                for jt in range(16):
                    js = slice(jt * 128, (jt + 1) * 128)
                    ps_sa = psS.tile([128, 1024], f32, tag="SA", bufs=1)
                    ps_sb = psS.tile([128, 1024], f32, tag="SB", bufs=1)
                    # heater
                    nc.tensor.matmul(
                        ps_sa[:, 0:512], qk_sb[:, 0, js],
                        qk_sb[:, 1, 0:512], start=True, stop=True)
                    for poff, ps_s in ((0, ps_sa), (64, ps_sb)):
                        for ic in range(2):
                            cs = slice(ic * 512, (ic + 1) * 512)
                            qs = slice(ih * 1024 + ic * 512,
                                       ih * 1024 + ic * 512 + 512)
                            nc.tensor.matmul(
                                ps_s[:, cs], qk_sb[poff:poff + D, 4 + p, js],
                                qk_sb[poff:poff + D, p, qs],
                                start=True, stop=True)
                    e_a = ep.tile([128, 1024], bf16, tag="EA")
                    e_b = ep.tile([128, 1024], bf16, tag="EB")
                    nc.scalar.activation(e_a[:], ps_sa[:], Exp, scale=SCALE)
                    nc.scalar.activation(e_b[:], ps_sb[:], Exp, scale=SCALE)
                    # second heater between the score and ctx groups
                    nc.tensor.matmul(
                        ps_sb[:, 0:512], qk_sb[:, 0, js],
                        qk_sb[:, 1, 0:512], start=True, stop=True)
                    for sidx, (ps_c, e_t) in enumerate(((ps_ca, e_a),
                                                        (ps_cb, e_b))):
                        for ic in range(2):
                            cs = slice(ic * 512, (ic + 1) * 512)
                            nc.tensor.matmul(
                                ps_c[:, cs], v_sb[:, jt, 2 * p + sidx, 0:D + 1],
                                e_t[:, cs], start=(jt == 0), stop=(jt == 15))
                # softmax division per head (denominator = row 64 of ctx psum)
                for s, ps_c in ((0, ps_ca), (1, ps_cb)):
                    den = dv.tile([65, 1024], f32r, tag="den", name=f"den{s}")
                    with nc.allow_low_precision(reason="f32r denom"):
                        nc.vector.tensor_copy(den[64:65, :], ps_c[64:65, :])
                    bc = psS.tile([64, 1024], f32, tag="SA", name=f"bc{s}",
                                  bufs=1)
                    for ic in range(2):
                        cs = slice(ic * 512, (ic + 1) * 512)
                        nc.tensor.matmul(bc[:, cs], ones_t[64:65, :],
                                         den[64:65, cs], start=True, stop=True)
                    rec = dv.tile([64, 1024], f32, tag="rec", name=f"rec{s}")
                    nc.vector.reciprocal_approx_fast(rec[:], bc[:])
                    with nc.allow_low_precision(reason="bf16 ctx"):
                        if s == 0:
                            nc.vector.tensor_tensor(
                                out=ctx_sb[0:D, p, ihs], in0=ps_c[0:D, :],
                                in1=rec[:], op=Mult)
                        else:
                            tmp = dv.tile([64, 1024], bf16, tag="tmp")
                            nc.vector.tensor_tensor(
                                out=tmp[:], in0=ps_c[0:D, :], in1=rec[:],
                                op=Mult)
                            # shift to partitions 64:128 via identity matmul
                            sh = psS.tile([128, 1024], f32, tag="SB",
                                          name="shift", bufs=1)
                            for ic in range(2):
                                cs = slice(ic * 512, (ic + 1) * 512)
                                nc.tensor.matmul(sh[64:128, cs], ident[:],
                                                 tmp[:, cs], start=True,
                                                 stop=True)
                            with nc.allow_low_precision(reason="bf16 ctx"):
                                nc.vector.tensor_copy(
                                    ctx_sb[64:128, p, ihs], sh[64:128, :])

    vp_.release()
    qkp.release()

    # ---- stage 3: output projection ----
    with (
        tc.tile_pool(name="w3", bufs=1) as w3,
        tc.tile_pool(name="w3s", bufs=2) as w3s,
        tc.tile_pool(name="psO", bufs=2, space="PSUM") as psO,
        tc.tile_pool(name="ot", bufs=3) as otp,
    ):
        wout_sb = w3.tile([128, NPAIR, DM], bf16, tag="wout")  # 8 KB/part
        for pb in range(NPAIR):
            wst3 = w3s.tile([128, DM], f32, tag="wst3")
            nc.sync.dma_start(wst3[:], woutT[pb * 128:(pb + 1) * 128, :])
            with nc.allow_low_precision(reason="bf16 cast"):
                nc.vector.tensor_copy(wout_sb[:, pb, :], wst3[:])
        for tt in range(16):
            o_t = otp.tile([128, DM], f32, tag="o")
            for fc in range(2):
                fs = slice(fc * 512, (fc + 1) * 512)
                ps = psO.tile([128, 512], f32, tag="O")
                for pb in range(NPAIR):
                    nc.tensor.matmul(
                        ps[:], ctx_sb[:, pb, tt * 128:(tt + 1) * 128],
                        wout_sb[:, pb, fs],
                        start=(pb == 0), stop=(pb == NPAIR - 1))
                nc.vector.tensor_copy(o_t[:, fs], ps[:])
            nc.sync.dma_start(out_p[tt * 128:(tt + 1) * 128, :], o_t[:])
    ctp.release()


_CACHE = {}


def _get_nc():
    if "nc" not in _CACHE:
        nc = bacc.Bacc("TRN2", target_bir_lowering=False, debug=False)
        xT = nc.dram_tensor("xT", [DM, TOK], f32, kind="ExternalInput").ap()
        wqkT = nc.dram_tensor("wqkT", [DM, 2 * DL], f32, kind="ExternalInput").ap()
        wvT = nc.dram_tensor("wvT", [DM, DL], f32, kind="ExternalInput").ap()
        woutT = nc.dram_tensor("woutT", [DL, DM], f32, kind="ExternalInput").ap()
        out_p = nc.dram_tensor("out_p", [TOK, DM], f32, kind="ExternalOutput").ap()
        with tile.TileContext(nc) as tc:
            _build(tc, xT, wqkT, wvT, woutT, out_p)
        nc.compile()
        _CACHE["nc"] = nc
    return _CACHE["nc"]


def make_in_maps(x, w_qkv, w_out):
    in_maps = []
    for c in range(N_CORES):
        b, g = c // 2, c % 2
        gs = slice(g * DL, (g + 1) * DL)
        wq = w_qkv[0 * DM + g * DL:0 * DM + (g + 1) * DL]
        wk = w_qkv[1 * DM + g * DL:1 * DM + (g + 1) * DL]
        wv = w_qkv[2 * DM + g * DL:2 * DM + (g + 1) * DL]
        in_maps.append({
            "xT": np.ascontiguousarray(x[b].T),
            "wqkT": np.ascontiguousarray(np.concatenate([wq, wk], 0).T),
            "wvT": np.ascontiguousarray(wv.T),
            "woutT": np.ascontiguousarray(w_out[:, gs].T),
        })
    return in_maps


def kernel(x, w_qkv, w_out, b_out, _trace=False):
    x = np.asarray(x, dtype=np.float32)
    w_qkv = np.asarray(w_qkv, dtype=np.float32)
    w_out = np.asarray(w_out, dtype=np.float32)
    b_out = np.asarray(b_out, dtype=np.float32)

    nc = _get_nc()
    in_maps = make_in_maps(x, w_qkv, w_out)
    res = bass_utils.run_bass_kernel_spmd(
        nc, in_maps, core_ids=list(range(N_CORES)), trace=_trace)
    out = np.empty((4, TOK, DM), dtype=np.float32)
    for b in range(4):
        out[b] = res.results[2 * b]["out_p"] + res.results[2 * b + 1]["out_p"]
    out += b_out
    if _trace:
        kernel.last_results = res
    return out



# revision 9
# speedup vs baseline: 1.2229x; 1.2229x over previous
"""Multi-head attention (b=4, n=2048, dm=1024, h=16) on 8 TRN2 NeuronCores.

Sharding: batch (4) x head-group (2) -> 8 cores, Megatron-style.
Core c handles batch c//2 and heads [8*(c%2), 8*(c%2)+8).

All matmul operands are bf16, converted on HOST (inputs DMA'd as bf16).

Per core:
  1. QKV projection (q,k transposed [feat, tok]; v natural [tok, feat] with a
     ones column appended per head for the softmax denominator).
  2. Attention over 16 blocks = (query-chunk ic of 512) x (head-pair p).
     Scores S^T = k^T q per 128-key block, the two heads of a pair packed at
     partitions 0:64 / 64:128 so their K=64 matmuls overlap via PE row groups.
     exp split across engines: ScalarE (exact spline exp, 15/32 tiles) and
     VectorE (Schraudolph bit-trick exp in bf16-bit space, 17/32 tiles;
     ~3% max rel err, cancels in the softmax ratio).
     ctx^T accumulated with a 64-wide ones block in the stationary v so the
     denominator comes out broadcast across the OTHER 64 partitions for free
     (M=128; matmul cost depends only on N).  Head a: [v|ones] -> ctx at
     rows 0:64, den at 64:128; head b: [ones|v] -> den at 0:64, ctx at
     64:128.  Division: fast reciprocal (lane-local), SBUF->SBUF DMA swaps
     the halves onto the ctx lanes, two multiplies.  Division tails are
     EMITTED one block late so they overlap the next block's compute and
     the PE never idles (keeps HAM clock at 2.4GHz).
  3. Output projection streamed per chunk; PSUM written straight to DRAM by
     DMA (no engine copy). Host sums the two partials per batch + bias.
"""

import numpy as np
import ml_dtypes

import concourse.bass as bass
import concourse.tile as tile
from concourse import bacc, mybir
from concourse import bass_utils

f32 = mybir.dt.float32
f32r = mybir.dt.float32r
bf16 = mybir.dt.bfloat16
i16 = mybir.dt.int16
Exp = mybir.ActivationFunctionType.Exp
Mult = mybir.AluOpType.mult
Add = mybir.AluOpType.add

TOK = 2048          # tokens per batch
DM = 1024           # model dim
DL = 512            # local q/k/v feature dim (8 heads x 64)
D = 64              # head dim
NH = 8              # local heads
NPAIR = 4           # head pairs (partition blocks of ctx/qk)
KT = 8              # dm / 128 contraction tiles
SCALE = DM ** (-0.5)
N_CORES = 8

# Schraudolph exp in bf16-bit space: bits = round(A*raw + B); value = bf16(bits)
# approximates exp(raw * SCALE).  C=5.5 minimizes max rel err (~3.3%).
A_SCH = 128.0 * 1.4426950408889634 * SCALE
B_SCH = 16256.0 - 5.5


def _build(tc, xT, wqkT, wvT, woutT, out_p, dbg=None):
    nc = tc.nc

    per = tc.alloc_tile_pool(name="per", bufs=1)
    qk_sb = per.tile([128, 8, TOK], bf16, tag="qk")        # 32 KB/part
    v_sb = per.tile([128, 16, NPAIR, 2, 128], bf16, tag="v")  # 32 KB/part
    ctx_sb = per.tile([128, NPAIR, TOK], bf16, tag="ctx")  # 16 KB/part
    wqk = [per.tile([128, 2 * DL], bf16, tag=f"wqk{kt}", name=f"wqk{kt}")
           for kt in range(KT)]
    wv = [per.tile([128, DL], bf16, tag=f"wv{kt}", name=f"wv{kt}")
          for kt in range(KT)]
    wout = [per.tile([128, DM], bf16, tag=f"wout{pb}", name=f"wout{pb}")
            for pb in range(NPAIR)]
    for kt in range(KT):
        nc.sync.dma_start(wqk[kt][:], wqkT[kt * 128:(kt + 1) * 128, :])
    for kt in range(KT):
        nc.sync.dma_start(wv[kt][:], wvT[kt * 128:(kt + 1) * 128, :])
    for pb in range(NPAIR):
        nc.sync.dma_start(wout[pb][:], woutT[pb * 128:(pb + 1) * 128, :])

    # ---- stage 1: QKV projection, token-quarters of 512 ----
    with (
        tc.tile_pool(name="x", bufs=16) as xp,
        tc.tile_pool(name="psq", bufs=4, space="PSUM") as psq,
    ):
        for tq in range(4):
            ts512 = slice(tq * 512, (tq + 1) * 512)
            jq = slice(tq * 4, (tq + 1) * 4)
            # ones blocks of v'' (denominator broadcast): head a cols 64:128,
            # head b cols 0:64
            nc.vector.memset(v_sb[:, jq, :, 0, D:128], 1.0)
            nc.vector.memset(v_sb[:, jq, :, 1, 0:D], 1.0)
            xq = []
            for kt in range(KT):
                xk = xp.tile([128, 512], bf16, tag="x")
                nc.sync.dma_start(xk[:], xT[kt * 128:(kt + 1) * 128, ts512])
                xq.append(xk)
            # q/k (transposed layout): out [feat 128, tok 512]
            for f in range(8):
                ps = psq.tile([128, 512], f32, tag="qk")
                for kt in range(KT):
                    nc.tensor.matmul(
                        ps[:], wqk[kt][:, f * 128:(f + 1) * 128],
                        xq[kt][:], start=(kt == 0), stop=(kt == KT - 1))
                with nc.allow_low_precision(reason="bf16"):
                    if f % 2 == 0:
                        nc.scalar.copy(qk_sb[:, f, ts512], ps[:])
                    else:
                        nc.vector.tensor_copy(qk_sb[:, f, ts512], ps[:])
            # v (natural layout): out [tok 128, feat 512]
            for tt in range(4):
                jt = tq * 4 + tt
                ps = psq.tile([128, 512], f32, tag="v")
                for kt in range(KT):
                    nc.tensor.matmul(
                        ps[:], xq[kt][:, tt * 128:(tt + 1) * 128],
                        wv[kt][:], start=(kt == 0), stop=(kt == KT - 1))
                with nc.allow_low_precision(reason="bf16 v"):
                    pv = ps[:].rearrange("p (h4 two d) -> p h4 two d", h4=4,
                                         two=2)
                    if tt % 2 == 0:
                        nc.vector.tensor_copy(v_sb[:, jt, :, 0, 0:D],
                                              pv[:, :, 0, :])
                        nc.scalar.copy(v_sb[:, jt, :, 1, D:128],
                                       pv[:, :, 1, :])
                    else:
                        nc.scalar.copy(v_sb[:, jt, :, 0, 0:D],
                                       pv[:, :, 0, :])
                        nc.vector.tensor_copy(v_sb[:, jt, :, 1, D:128],
                                              pv[:, :, 1, :])

    # ---- stage 2+3: attention blocks (query-chunk x head-pair), deferred
    # division tails, out-projection streamed per chunk ----
    with (
        tc.tile_pool(name="psS", bufs=4, space="PSUM") as psS,
        tc.tile_pool(name="psC", bufs=4, space="PSUM") as psC,
        tc.tile_pool(name="ep", bufs=4) as ep,
        tc.tile_pool(name="dv", bufs=2) as dv,
        tc.tile_pool(name="ot", bufs=4) as otp,
    ):
        blocks = [(ic, p) for ic in range(4) for p in range(4)]
        state = {}

        def emit_jt_loop(bi):
            ic, p = blocks[bi]
            isl = slice(ic * 512, (ic + 1) * 512)
            ps_ca = psC.tile([128, 512], f32, tag="C", name=f"ca{bi}")
            ps_cb = psC.tile([128, 512], f32, tag="C", name=f"cb{bi}")
            for jt in range(16):
                js = slice(jt * 128, (jt + 1) * 128)
                ps_sa = psS.tile([128, 512], f32, tag="S", name=f"sa{bi}_{jt}")
                ps_sb = psS.tile([128, 512], f32, tag="S", name=f"sb{bi}_{jt}")
                nc.tensor.matmul(ps_sa[:], qk_sb[0:D, 4 + p, js],
                                 qk_sb[0:D, p, isl], start=True, stop=True)
                nc.tensor.matmul(ps_sb[:], qk_sb[64:64 + D, 4 + p, js],
                                 qk_sb[64:64 + D, p, isl], start=True,
                                 stop=True)
                e_a = ep.tile([128, 512], bf16, tag="EA")
                e_b = ep.tile([128, 512], bf16, tag="EB")
                with nc.allow_low_precision(reason="schraudolph exp"):
                    if jt == 7:
                        nc.vector.tensor_scalar(
                            out=e_a[:].bitcast(i16), in0=ps_sa[:],
                            scalar1=A_SCH, scalar2=B_SCH, op0=Mult, op1=Add)
                    else:
                        nc.scalar.activation(e_a[:], ps_sa[:], Exp,
                                             scale=SCALE)
                    nc.vector.tensor_scalar(
                        out=e_b[:].bitcast(i16), in0=ps_sb[:],
                        scalar1=A_SCH, scalar2=B_SCH, op0=Mult, op1=Add)
                # head a: [v|ones] -> ctx rows 0:64, den rows 64:128
                # head b: [ones|v] -> den rows 0:64, ctx rows 64:128
                nc.tensor.matmul(ps_ca[:], v_sb[:, jt, p, 0, :], e_a[:],
                                 start=(jt == 0), stop=(jt == 15))
                nc.tensor.matmul(ps_cb[:], v_sb[:, jt, p, 1, :], e_b[:],
                                 start=(jt == 0), stop=(jt == 15))
            # den halves -> SBUF (lane-local copies), DMA swaps them onto
            # the ctx lanes; reciprocal runs later at base partition 0
            # (reciprocal_approx_fast ucode misbehaves at base partition 64)
            db = dv.tile([128, 512], f32, tag="db", name=f"db{bi}")
            rs = dv.tile([128, 512], f32, tag="rs", name=f"rs{bi}")
            nc.vector.tensor_copy(db[64:128, :], ps_ca[64:128, :])
            nc.vector.tensor_copy(db[0:64, :], ps_cb[0:64, :])
            nc.sync.dma_start(rs[0:64, :], db[64:128, :])
            nc.sync.dma_start(rs[64:128, :], db[0:64, :])
            state[bi] = (ps_ca, ps_cb, rs)

        def emit_div_mults(bi):
            ic, p = blocks[bi]
            isl = slice(ic * 512, (ic + 1) * 512)
            ps_ca, ps_cb, rs = state.pop(bi)
            rec = dv.tile([128, 512], f32, tag="rec", name=f"rec{bi}")
            nc.vector.reciprocal_approx_fast(rec[:], rs[:])
            with nc.allow_low_precision(reason="bf16 ctx"):
                nc.vector.tensor_tensor(
                    out=ctx_sb[0:64, p, isl], in0=ps_ca[0:64, :],
                    in1=rec[0:64, :], op=Mult)
                nc.vector.tensor_tensor(
                    out=ctx_sb[64:128, p, isl], in0=ps_cb[64:128, :],
                    in1=rec[64:128, :], op=Mult)

        def emit_stage3(ic):
            for tt in range(4):
                tb = ic * 4 + tt
                tsl = slice(tb * 128, (tb + 1) * 128)
                for fc in range(2):
                    fs = slice(fc * 512, (fc + 1) * 512)
                    ps = psS.tile([128, 512], f32, tag="S",
                                  name=f"o{tb}_{fc}")
                    for pb in range(NPAIR):
                        nc.tensor.matmul(
                            ps[:], ctx_sb[:, pb, tsl], wout[pb][:, fs],
                            start=(pb == 0), stop=(pb == NPAIR - 1))
                    o_t = otp.tile([128, 512], f32, tag="o",
                                   name=f"ot{tb}_{fc}")
                    if fc == 0:
                        nc.scalar.copy(o_t[:], ps[:])
                    else:
                        nc.vector.tensor_copy(o_t[:], ps[:])
                    nc.sync.dma_start(out_p[tsl, fs], o_t[:])

        def emit_dbg():
            if dbg is None:
                return
            ps_ca, ps_cb, rs = state[0]
            cpa = dv.tile([128, 512], f32, tag="cpa", name="cpa")
            cpb = dv.tile([128, 512], f32, tag="cpb", name="cpb")
            nc.vector.tensor_copy(cpa[:], ps_ca[:])
            nc.vector.tensor_copy(cpb[:], ps_cb[:])
            nc.sync.dma_start(dbg["ca0"][:, :], cpa[:])
            nc.sync.dma_start(dbg["cb0"][:, :], cpb[:])
            nc.sync.dma_start(dbg["rs0"][:, :], rs[:])

        for bi in range(len(blocks) + 2):
            if bi < len(blocks):
                emit_jt_loop(bi)
            if bi == 1 and dbg is not None:
                emit_dbg()
            if 0 <= bi - 1 < len(blocks):
                emit_div_mults(bi - 1)
            if bi - 2 >= 0 and blocks[bi - 2][1] == NPAIR - 1:
                emit_stage3(blocks[bi - 2][0])

        if dbg is not None:
            nc.sync.dma_start(dbg["qk"][:, :], qk_sb[:].rearrange("p a b -> p (a b)"))
            nc.sync.dma_start(dbg["v"][:, :], v_sb[:].rearrange("p a b c d -> p (a b c d)"))
            nc.sync.dma_start(dbg["ctx"][:, :], ctx_sb[:].rearrange("p a b -> p (a b)"))

    per.release()


_CACHE = {}


def _get_nc(debug=False):
    key = "nc_dbg" if debug else "nc"
    if key not in _CACHE:
        nc = bacc.Bacc("TRN2", target_bir_lowering=False, debug=False)
        xT = nc.dram_tensor("xT", [DM, TOK], bf16, kind="ExternalInput").ap()
        wqkT = nc.dram_tensor("wqkT", [DM, 2 * DL], bf16, kind="ExternalInput").ap()
        wvT = nc.dram_tensor("wvT", [DM, DL], bf16, kind="ExternalInput").ap()
        woutT = nc.dram_tensor("woutT", [DL, DM], bf16, kind="ExternalInput").ap()
        out_p = nc.dram_tensor("out_p", [TOK, DM], f32, kind="ExternalOutput").ap()
        dbg = None
        if debug:
            dbg = {
                "qk": nc.dram_tensor("qk", [128, 8 * TOK], bf16, kind="ExternalOutput").ap(),
                "v": nc.dram_tensor("v", [128, 16 * NPAIR * 2 * 128], bf16, kind="ExternalOutput").ap(),
                "ctx": nc.dram_tensor("ctx", [128, NPAIR * TOK], bf16, kind="ExternalOutput").ap(),
                "ca0": nc.dram_tensor("ca0", [128, 512], f32, kind="ExternalOutput").ap(),
                "cb0": nc.dram_tensor("cb0", [128, 512], f32, kind="ExternalOutput").ap(),
                "rs0": nc.dram_tensor("rs0", [128, 512], f32, kind="ExternalOutput").ap(),
            }
        with tile.TileContext(nc) as tc:
            _build(tc, xT, wqkT, wvT, woutT, out_p, dbg=dbg)
        nc.compile()
        _CACHE[key] = nc
    return _CACHE[key]


def make_in_maps(x, w_qkv, w_out):
    bf = ml_dtypes.bfloat16
    in_maps = []
    for c in range(N_CORES):
        b, g = c // 2, c % 2
        gs = slice(g * DL, (g + 1) * DL)
        wq = w_qkv[0 * DM + g * DL:0 * DM + (g + 1) * DL]
        wk = w_qkv[1 * DM + g * DL:1 * DM + (g + 1) * DL]
        wv = w_qkv[2 * DM + g * DL:2 * DM + (g + 1) * DL]
        in_maps.append({
            "xT": np.ascontiguousarray(x[b].T).astype(bf),
            "wqkT": np.ascontiguousarray(np.concatenate([wq, wk], 0).T).astype(bf),
            "wvT": np.ascontiguousarray(wv.T).astype(bf),
            "woutT": np.ascontiguousarray(w_out[:, gs].T).astype(bf),
        })
    return in_maps


def kernel(x, w_qkv, w_out, b_out, _trace=False):
    x = np.asarray(x, dtype=np.float32)
    w_qkv = np.asarray(w_qkv, dtype=np.float32)
    w_out = np.asarray(w_out, dtype=np.float32)
    b_out = np.asarray(b_out, dtype=np.float32)

    nc = _get_nc()
    in_maps = make_in_maps(x, w_qkv, w_out)
    res = bass_utils.run_bass_kernel_spmd(
        nc, in_maps, core_ids=list(range(N_CORES)), trace=_trace)
    out = np.empty((4, TOK, DM), dtype=np.float32)
    for b in range(4):
        out[b] = res.results[2 * b]["out_p"] + res.results[2 * b + 1]["out_p"]
    out += b_out
    if _trace:
        kernel.last_results = res
    return out


# revision 11
# speedup vs baseline: 1.3949x; 1.1406x over previous
"""Multi-head attention (b=4, n=2048, dm=1024, h=16) on 8 TRN2 NeuronCores.

Sharding: batch (4) x head-group (2) -> 8 cores, Megatron-style.
Core c handles batch c//2 and heads [8*(c%2), 8*(c%2)+8).

All matmul operands are bf16, converted on HOST (inputs DMA'd as bf16).

Per core:
  1. QKV projection (q,k transposed [feat, tok]; v natural [tok, feat] with a
     ones column appended per head for the softmax denominator).
  2. Attention over 16 blocks = (query-chunk ic of 512) x (head-pair p).
     Scores S^T = k^T q per 128-key block, the two heads of a pair packed at
     partitions 0:64 / 64:128 so their K=64 matmuls overlap via PE row groups.
     exp split across engines: ScalarE (exact spline exp, 15/32 tiles) and
     VectorE (Schraudolph bit-trick exp in bf16-bit space, 17/32 tiles;
     ~3% max rel err, cancels in the softmax ratio).
     ctx^T accumulated with a 64-wide ones block in the stationary v so the
     denominator comes out broadcast across the OTHER 64 partitions for free
     (M=128; matmul cost depends only on N).  Head a: [v|ones] -> ctx at
     rows 0:64, den at 64:128; head b: [ones|v] -> den at 0:64, ctx at
     64:128.  Division: fast reciprocal (lane-local), SBUF->SBUF DMA swaps
     the halves onto the ctx lanes, two multiplies.  Division tails are
     EMITTED one block late so they overlap the next block's compute and
     the PE never idles (keeps HAM clock at 2.4GHz).
  3. Output projection streamed per chunk; PSUM written straight to DRAM by
     DMA (no engine copy). Host sums the two partials per batch + bias.
"""

import numpy as np
import ml_dtypes

import concourse.bass as bass
import concourse.tile as tile
from concourse import bacc, mybir
from concourse import bass_utils

f32 = mybir.dt.float32
f32r = mybir.dt.float32r
bf16 = mybir.dt.bfloat16
i16 = mybir.dt.int16
Exp = mybir.ActivationFunctionType.Exp
Mult = mybir.AluOpType.mult
Add = mybir.AluOpType.add

TOK = 2048          # tokens per batch
DM = 1024           # model dim
DL = 512            # local q/k/v feature dim (8 heads x 64)
D = 64              # head dim
NH = 8              # local heads
NPAIR = 4           # head pairs (partition blocks of ctx/qk)
KT = 8              # dm / 128 contraction tiles
SCALE = DM ** (-0.5)
N_CORES = 8

# Schraudolph exp in bf16-bit space: bits = round(A*raw + B); value = bf16(bits)
# approximates exp(raw * SCALE).  C=5.5 minimizes max rel err (~3.3%).
A_SCH = 128.0 * 1.4426950408889634 * SCALE
B_SCH = 16256.0 - 5.5


def _build(tc, xT, wqkT, wvT, woutT, out_p, dbg=None):
    nc = tc.nc

    per = tc.alloc_tile_pool(name="per", bufs=1)
    qk_sb = per.tile([128, 8, TOK], bf16, tag="qk")        # 32 KB/part
    v_sb = per.tile([128, 16, NPAIR, 2, 128], bf16, tag="v")  # 32 KB/part
    ctx_sb = per.tile([128, NPAIR, TOK], bf16, tag="ctx")  # 16 KB/part
    wqk = [per.tile([128, 2 * DL], bf16, tag=f"wqk{kt}", name=f"wqk{kt}")
           for kt in range(KT)]
    wv = [per.tile([128, DL], bf16, tag=f"wv{kt}", name=f"wv{kt}")
          for kt in range(KT)]
    wout = [per.tile([128, DM], bf16, tag=f"wout{pb}", name=f"wout{pb}")
            for pb in range(NPAIR)]
    for kt in range(KT):
        nc.sync.dma_start(wqk[kt][:], wqkT[kt * 128:(kt + 1) * 128, :])
    for kt in range(KT):
        nc.sync.dma_start(wv[kt][:], wvT[kt * 128:(kt + 1) * 128, :])
    for pb in range(NPAIR):
        nc.sync.dma_start(wout[pb][:], woutT[pb * 128:(pb + 1) * 128, :])

    # ---- PE warmup: junk matmuls on uninitialized SBUF during the initial
    # DMA wait, so HAM un-throttles the clock before real work arrives ----
    with (
        tc.tile_pool(name="wu", bufs=1) as wup,
        tc.tile_pool(name="pswu", bufs=1, space="PSUM") as pswu,
    ):
        wtile = wup.tile([128, 512], bf16, tag="wt")
        wps = pswu.tile([128, 512], f32, tag="wps")
        nc.gpsimd.memset(wtile[:], 1.0)
        for _ in range(16):
            nc.tensor.matmul(wps[:], wtile[:, 0:128], wtile[:],
                             start=True, stop=True)

    # ---- stage 1: QKV projection, token-quarters of 512 ----
    with (
        tc.tile_pool(name="x", bufs=16) as xp,
        tc.tile_pool(name="psq", bufs=4, space="PSUM") as psq,
    ):
        for tq in range(4):
            ts512 = slice(tq * 512, (tq + 1) * 512)
            jq = slice(tq * 4, (tq + 1) * 4)
            # ones blocks of v'' (denominator broadcast): head a cols 64:128,
            # head b cols 0:64
            nc.vector.memset(v_sb[:, jq, :, 0, D:128], 1.0)
            nc.vector.memset(v_sb[:, jq, :, 1, 0:D], 1.0)
            xq = []
            for kt in range(KT):
                xk = xp.tile([128, 512], bf16, tag="x")
                nc.sync.dma_start(xk[:], xT[kt * 128:(kt + 1) * 128, ts512])
                xq.append(xk)
            # q/k (transposed layout): out [feat 128, tok 512]
            for f in range(8):
                ps = psq.tile([128, 512], f32, tag="qk")
                for kt in range(KT):
                    nc.tensor.matmul(
                        ps[:], wqk[kt][:, f * 128:(f + 1) * 128],
                        xq[kt][:], start=(kt == 0), stop=(kt == KT - 1))
                with nc.allow_low_precision(reason="bf16"):
                    if f % 2 == 0:
                        nc.scalar.copy(qk_sb[:, f, ts512], ps[:])
                    else:
                        nc.vector.tensor_copy(qk_sb[:, f, ts512], ps[:])
            # v (natural layout): out [tok 128, feat 512]
            for tt in range(4):
                jt = tq * 4 + tt
                ps = psq.tile([128, 512], f32, tag="v")
                for kt in range(KT):
                    nc.tensor.matmul(
                        ps[:], xq[kt][:, tt * 128:(tt + 1) * 128],
                        wv[kt][:], start=(kt == 0), stop=(kt == KT - 1))
                with nc.allow_low_precision(reason="bf16 v"):
                    pv = ps[:].rearrange("p (h4 two d) -> p h4 two d", h4=4,
                                         two=2)
                    if tt % 2 == 0:
                        nc.vector.tensor_copy(v_sb[:, jt, :, 0, 0:D],
                                              pv[:, :, 0, :])
                        nc.scalar.copy(v_sb[:, jt, :, 1, D:128],
                                       pv[:, :, 1, :])
                    else:
                        nc.scalar.copy(v_sb[:, jt, :, 0, 0:D],
                                       pv[:, :, 0, :])
                        nc.vector.tensor_copy(v_sb[:, jt, :, 1, D:128],
                                              pv[:, :, 1, :])

    # ---- stage 2+3: attention blocks (query-chunk x head-pair), jt-paired
    # PSUM tiles (1024-wide exp ops), ctx delayed one pair so the in-order PE
    # queue never waits on exp, deferred division, out-proj per chunk ----
    with (
        tc.tile_pool(name="psS", bufs=1, space="PSUM") as psS,
        tc.tile_pool(name="psC", bufs=4, space="PSUM") as psC,
        tc.tile_pool(name="ep", bufs=2) as ep,
        tc.tile_pool(name="dv", bufs=2) as dv,
        tc.tile_pool(name="ot", bufs=4) as otp,
    ):
        blocks = [(ic, p) for ic in range(4) for p in range(4)]
        state = {}

        def emit_jt_loop(bi):
            ic, p = blocks[bi]
            isl = slice(ic * 512, (ic + 1) * 512)
            ps_ca = psC.tile([128, 512], f32, tag="C", name=f"ca{bi}")
            ps_cb = psC.tile([128, 512], f32, tag="C", name=f"cb{bi}")
            pend = None

            def emit_ctx(k, ea2, eb2):
                for j2 in range(2):
                    jt = 2 * k + j2
                    es = slice(j2 * 512, (j2 + 1) * 512)
                    st = (jt == 0)
                    sp = (jt == 15)
                    nc.tensor.matmul(ps_ca[:], v_sb[:, jt, p, 0, :],
                                     ea2[:, es], start=st, stop=sp)
                    nc.tensor.matmul(ps_cb[:], v_sb[:, jt, p, 1, :],
                                     eb2[:, es], start=st, stop=sp)

            for k in range(8):
                sa2 = psS.tile([128, 1024], f32, tag="SA2", name=f"sa{bi}_{k}")
                sb2 = psS.tile([128, 1024], f32, tag="SB2", name=f"sb{bi}_{k}")
                for j2 in range(2):
                    jt = 2 * k + j2
                    js = slice(jt * 128, (jt + 1) * 128)
                    es = slice(j2 * 512, (j2 + 1) * 512)
                    nc.tensor.matmul(sa2[:, es], qk_sb[0:D, 4 + p, js],
                                     qk_sb[0:D, p, isl], start=True, stop=True)
                    nc.tensor.matmul(sb2[:, es], qk_sb[64:64 + D, 4 + p, js],
                                     qk_sb[64:64 + D, p, isl], start=True,
                                     stop=True)
                ea2 = ep.tile([128, 1024], bf16, tag="EA")
                eb2 = ep.tile([128, 1024], bf16, tag="EB")
                with nc.allow_low_precision(reason="schraudolph exp"):
                    nc.scalar.activation(ea2[:], sa2[:], Exp, scale=SCALE)
                    nc.vector.tensor_scalar(
                        out=eb2[:].bitcast(i16), in0=sb2[:],
                        scalar1=A_SCH, scalar2=B_SCH, op0=Mult, op1=Add)
                if pend is not None:
                    emit_ctx(k - 1, *pend)
                pend = (ea2, eb2)
            emit_ctx(7, *pend)
            # den halves -> SBUF (ACT), DMA swaps them onto the ctx lanes;
            # reciprocal later at base partition 0 (custom ucode misbehaves
            # at base partition 64)
            db = dv.tile([128, 512], f32, tag="db", name=f"db{bi}")
            rs = dv.tile([128, 512], f32, tag="rs", name=f"rs{bi}")
            nc.scalar.copy(db[64:128, :], ps_ca[64:128, :])
            nc.scalar.copy(db[0:64, :], ps_cb[0:64, :])
            nc.sync.dma_start(rs[0:64, :], db[64:128, :])
            nc.sync.dma_start(rs[64:128, :], db[0:64, :])
            state[bi] = (ps_ca, ps_cb, rs)

        def emit_div_mults(bi):
            ic, p = blocks[bi]
            isl = slice(ic * 512, (ic + 1) * 512)
            ps_ca, ps_cb, rs = state.pop(bi)
            rec = dv.tile([128, 512], f32, tag="rec", name=f"rec{bi}")
            nc.vector.reciprocal_approx_fast(rec[:], rs[:])
            with nc.allow_low_precision(reason="bf16 ctx"):
                nc.vector.tensor_tensor(
                    out=ctx_sb[0:64, p, isl], in0=ps_ca[0:64, :],
                    in1=rec[0:64, :], op=Mult)
                nc.vector.tensor_tensor(
                    out=ctx_sb[64:128, p, isl], in0=ps_cb[64:128, :],
                    in1=rec[64:128, :], op=Mult)

        def emit_stage3(ic):
            for tt in range(4):
                tb = ic * 4 + tt
                tsl = slice(tb * 128, (tb + 1) * 128)
                for fc in range(2):
                    fs = slice(fc * 512, (fc + 1) * 512)
                    ps = psC.tile([128, 512], f32, tag="C",
                                  name=f"o{tb}_{fc}")
                    for pb in range(NPAIR):
                        nc.tensor.matmul(
                            ps[:], ctx_sb[:, pb, tsl], wout[pb][:, fs],
                            start=(pb == 0), stop=(pb == NPAIR - 1))
                    o_t = otp.tile([128, 512], f32, tag="o",
                                   name=f"ot{tb}_{fc}")
                    with nc.allow_low_precision(reason="copy"):
                        if fc == 0:
                            nc.scalar.copy(o_t[:], ps[:])
                        else:
                            nc.vector.tensor_copy(o_t[:], ps[:])
                    nc.sync.dma_start(out_p[tsl, fs], o_t[:])

        for bi in range(len(blocks) + 2):
            if bi < len(blocks):
                emit_jt_loop(bi)
            if 0 <= bi - 1 < len(blocks):
                emit_div_mults(bi - 1)
            if bi - 2 >= 0 and blocks[bi - 2][1] == NPAIR - 1:
                emit_stage3(blocks[bi - 2][0])

    per.release()


_CACHE = {}


def _get_nc(debug=False):
    key = "nc_dbg" if debug else "nc"
    if key not in _CACHE:
        nc = bacc.Bacc("TRN2", target_bir_lowering=False, debug=False)
        xT = nc.dram_tensor("xT", [DM, TOK], bf16, kind="ExternalInput").ap()
        wqkT = nc.dram_tensor("wqkT", [DM, 2 * DL], bf16, kind="ExternalInput").ap()
        wvT = nc.dram_tensor("wvT", [DM, DL], bf16, kind="ExternalInput").ap()
        woutT = nc.dram_tensor("woutT", [DL, DM], bf16, kind="ExternalInput").ap()
        out_p = nc.dram_tensor("out_p", [TOK, DM], f32, kind="ExternalOutput").ap()
        dbg = None
        if debug:
            dbg = {
                "qk": nc.dram_tensor("qk", [128, 8 * TOK], bf16, kind="ExternalOutput").ap(),
                "v": nc.dram_tensor("v", [128, 16 * NPAIR * 2 * 128], bf16, kind="ExternalOutput").ap(),
                "ctx": nc.dram_tensor("ctx", [128, NPAIR * TOK], bf16, kind="ExternalOutput").ap(),
                "ca0": nc.dram_tensor("ca0", [128, 512], f32, kind="ExternalOutput").ap(),
                "cb0": nc.dram_tensor("cb0", [128, 512], f32, kind="ExternalOutput").ap(),
                "rs0": nc.dram_tensor("rs0", [128, 512], f32, kind="ExternalOutput").ap(),
            }
        with tile.TileContext(nc) as tc:
            _build(tc, xT, wqkT, wvT, woutT, out_p, dbg=dbg)
        nc.compile()
        _CACHE[key] = nc
    return _CACHE[key]


def make_in_maps(x, w_qkv, w_out):
    bf = ml_dtypes.bfloat16
    in_maps = []
    for c in range(N_CORES):
        b, g = c // 2, c % 2
        gs = slice(g * DL, (g + 1) * DL)
        wq = w_qkv[0 * DM + g * DL:0 * DM + (g + 1) * DL]
        wk = w_qkv[1 * DM + g * DL:1 * DM + (g + 1) * DL]
        wv = w_qkv[2 * DM + g * DL:2 * DM + (g + 1) * DL]
        in_maps.append({
            "xT": np.ascontiguousarray(x[b].T).astype(bf),
            "wqkT": np.ascontiguousarray(np.concatenate([wq, wk], 0).T).astype(bf),
            "wvT": np.ascontiguousarray(wv.T).astype(bf),
            "woutT": np.ascontiguousarray(w_out[:, gs].T).astype(bf),
        })
    return in_maps


def kernel(x, w_qkv, w_out, b_out, _trace=False):
    x = np.asarray(x, dtype=np.float32)
    w_qkv = np.asarray(w_qkv, dtype=np.float32)
    w_out = np.asarray(w_out, dtype=np.float32)
    b_out = np.asarray(b_out, dtype=np.float32)

    nc = _get_nc()
    in_maps = make_in_maps(x, w_qkv, w_out)
    res = bass_utils.run_bass_kernel_spmd(
        nc, in_maps, core_ids=list(range(N_CORES)), trace=_trace)
    out = np.empty((4, TOK, DM), dtype=np.float32)
    for b in range(4):
        out[b] = res.results[2 * b]["out_p"] + res.results[2 * b + 1]["out_p"]
    out += b_out
    if _trace:
        kernel.last_results = res
    return out


# revision 18
# speedup vs baseline: 1.6188x; 1.1605x over previous
"""Multi-head attention (b=4, n=2048, dm=1024, h=16) on 8 TRN2 NeuronCores.

Sharding: batch (4) x head-group (2) -> 8 cores, Megatron-style.
Core c handles batch c//2 and heads [8*(c%2), 8*(c%2)+8).

All matmul operands are bf16, converted on HOST (inputs DMA'd as bf16).

Per core:
  1. QKV projection (q,k transposed [feat, tok]; v natural [tok, feat] with a
     ones column appended per head for the softmax denominator).
  2. Attention over 16 blocks = (query-chunk ic of 512) x (head-pair p).
     Scores S^T = k^T q per 128-key block, the two heads of a pair packed at
     partitions 0:64 / 64:128 so their K=64 matmuls overlap via PE row groups.
     exp split across engines: ScalarE (exact spline exp, 15/32 tiles) and
     VectorE (Schraudolph bit-trick exp in bf16-bit space, 17/32 tiles;
     ~3% max rel err, cancels in the softmax ratio).
     ctx^T accumulated with a 64-wide ones block in the stationary v so the
     denominator comes out broadcast across the OTHER 64 partitions for free
     (M=128; matmul cost depends only on N).  Head a: [v|ones] -> ctx at
     rows 0:64, den at 64:128; head b: [ones|v] -> den at 0:64, ctx at
     64:128.  Division: fast reciprocal (lane-local), SBUF->SBUF DMA swaps
     the halves onto the ctx lanes, two multiplies.  Division tails are
     EMITTED one block late so they overlap the next block's compute and
     the PE never idles (keeps HAM clock at 2.4GHz).
  3. Output projection streamed per chunk; PSUM written straight to DRAM by
     DMA (no engine copy). Host sums the two partials per batch + bias.
"""

import numpy as np
import ml_dtypes

import concourse.bass as bass
import concourse.tile as tile
from concourse import bacc, mybir
from concourse import bass_utils

f32 = mybir.dt.float32
f32r = mybir.dt.float32r
bf16 = mybir.dt.bfloat16
i16 = mybir.dt.int16
Exp = mybir.ActivationFunctionType.Exp
Mult = mybir.AluOpType.mult
Add = mybir.AluOpType.add

TOK = 2048          # tokens per batch
DM = 1024           # model dim
DL = 512            # local q/k/v feature dim (8 heads x 64)
D = 64              # head dim
NH = 8              # local heads
NPAIR = 4           # head pairs (partition blocks of ctx/qk)
KT = 8              # dm / 128 contraction tiles
SCALE = DM ** (-0.5)
N_CORES = 8

# Schraudolph exp in bf16-bit space: bits = round(A*raw + B); value = bf16(bits)
# approximates exp(raw * SCALE).  C=5.5 minimizes max rel err (~3.3%).
A_SCH = 128.0 * 1.4426950408889634 * SCALE
B_SCH = 16256.0 - 5.5


def _build(tc, xT, wqkT, wvT, woutT, out_p, dbg=None):
    nc = tc.nc

    per = tc.alloc_tile_pool(name="per", bufs=1)
    qk_sb = per.tile([128, 8, TOK], bf16, tag="qk")        # 32 KB/part
    v_sb = per.tile([128, 16, NPAIR, 2, 128], bf16, tag="v")  # 32 KB/part
    ctx_sb = per.tile([128, NPAIR, TOK], bf16, tag="ctx")  # 16 KB/part
    wqk = [per.tile([128, 2 * DL], bf16, tag=f"wqk{kt}", name=f"wqk{kt}")
           for kt in range(KT)]
    wv = [per.tile([128, DL], bf16, tag=f"wv{kt}", name=f"wv{kt}")
          for kt in range(KT)]
    wout = [per.tile([128, DM], bf16, tag=f"wout{pb}", name=f"wout{pb}")
            for pb in range(NPAIR)]
    for kt in range(KT):
        nc.sync.dma_start(wqk[kt][:], wqkT[kt * 128:(kt + 1) * 128, :])
    for kt in range(KT):
        nc.sync.dma_start(wv[kt][:], wvT[kt * 128:(kt + 1) * 128, :])
    for pb in range(NPAIR):
        nc.sync.dma_start(wout[pb][:], woutT[pb * 128:(pb + 1) * 128, :])

    # ---- PE warmup: junk matmuls on uninitialized SBUF during the initial
    # DMA wait, so HAM un-throttles the clock before real work arrives ----
    with (
        tc.tile_pool(name="wu", bufs=1) as wup,
        tc.tile_pool(name="pswu", bufs=1, space="PSUM") as pswu,
    ):
        wtile = wup.tile([128, 512], bf16, tag="wt")
        wps = pswu.tile([128, 512], f32, tag="wps")
        nc.gpsimd.memset(wtile[:], 1.0)
        for _ in range(30):
            nc.tensor.matmul(wps[:], wtile[:, 0:128], wtile[:],
                             start=True, stop=True)

    # ---- stage 1: QKV projection, token-quarters of 512 ----
    with (
        tc.tile_pool(name="x", bufs=16) as xp,
        tc.tile_pool(name="psq", bufs=4, space="PSUM") as psq,
    ):
        for tq in range(4):
            ts512 = slice(tq * 512, (tq + 1) * 512)
            jq = slice(tq * 4, (tq + 1) * 4)
            # ones blocks of v'' (denominator broadcast): head a cols 64:128,
            # head b cols 0:64
            nc.vector.memset(v_sb[:, jq, :, 0, D:128], 1.0)
            nc.vector.memset(v_sb[:, jq, :, 1, 0:D], 1.0)
            xq = []
            for kt in range(KT):
                xk = xp.tile([128, 512], bf16, tag="x")
                nc.sync.dma_start(xk[:], xT[kt * 128:(kt + 1) * 128, ts512])
                xq.append(xk)
            # q/k (transposed layout): out [feat 128, tok 512]
            for f in range(8):
                ps = psq.tile([128, 512], f32, tag="qk")
                for kt in range(KT):
                    nc.tensor.matmul(
                        ps[:], wqk[kt][:, f * 128:(f + 1) * 128],
                        xq[kt][:], start=(kt == 0), stop=(kt == KT - 1))
                with nc.allow_low_precision(reason="bf16"):
                    if f % 2 == 0:
                        nc.scalar.copy(qk_sb[:, f, ts512], ps[:])
                    else:
                        nc.vector.tensor_copy(qk_sb[:, f, ts512], ps[:])
            # v (natural layout): out [tok 128, feat 512]
            for tt in range(4):
                jt = tq * 4 + tt
                ps = psq.tile([128, 512], f32, tag="v")
                for kt in range(KT):
                    nc.tensor.matmul(
                        ps[:], xq[kt][:, tt * 128:(tt + 1) * 128],
                        wv[kt][:], start=(kt == 0), stop=(kt == KT - 1))
                with nc.allow_low_precision(reason="bf16 v"):
                    pv = ps[:].rearrange("p (h4 two d) -> p h4 two d", h4=4,
                                         two=2)
                    if tt % 2 == 0:
                        nc.vector.tensor_copy(v_sb[:, jt, :, 0, 0:D],
                                              pv[:, :, 0, :])
                        nc.scalar.copy(v_sb[:, jt, :, 1, D:128],
                                       pv[:, :, 1, :])
                    else:
                        nc.scalar.copy(v_sb[:, jt, :, 0, 0:D],
                                       pv[:, :, 0, :])
                        nc.vector.tensor_copy(v_sb[:, jt, :, 1, D:128],
                                              pv[:, :, 1, :])

    # ---- stage 2+3: attention blocks (query-chunk x head-pair), jt-paired
    # PSUM tiles (1024-wide exp ops), ctx delayed one pair so the in-order PE
    # queue never waits on exp, deferred division, out-proj per chunk ----
    with (
        tc.tile_pool(name="psS", bufs=1, space="PSUM") as psS,
        tc.tile_pool(name="psC", bufs=4, space="PSUM") as psC,
        tc.tile_pool(name="ep", bufs=2) as ep,
        tc.tile_pool(name="dv", bufs=2) as dv,
        tc.tile_pool(name="ot", bufs=4) as otp,
    ):
        blocks = [(ic, p) for ic in range(4) for p in range(4)]
        state = {}

        def emit_jt_loop(bi):
            ic, p = blocks[bi]
            isl = slice(ic * 512, (ic + 1) * 512)
            ps_ca = psC.tile([128, 512], f32, tag="C", name=f"ca{bi}")
            ps_cb = psC.tile([128, 512], f32, tag="C", name=f"cb{bi}")
            pend = None

            def emit_ctx(k, ea2, eb2):
                for j2 in range(2):
                    jt = 2 * k + j2
                    es = slice(j2 * 512, (j2 + 1) * 512)
                    st = (jt == 0)
                    sp = (jt == 15)
                    nc.tensor.matmul(ps_ca[:], v_sb[:, jt, p, 0, :],
                                     ea2[:, es], start=st, stop=sp)
                    nc.tensor.matmul(ps_cb[:], v_sb[:, jt, p, 1, :],
                                     eb2[:, es], start=st, stop=sp)

            for k in range(8):
                sa2 = psS.tile([128, 1024], f32, tag="SA2", name=f"sa{bi}_{k}")
                sb2 = psS.tile([128, 1024], f32, tag="SB2", name=f"sb{bi}_{k}")
                for j2 in range(2):
                    jt = 2 * k + j2
                    js = slice(jt * 128, (jt + 1) * 128)
                    es = slice(j2 * 512, (j2 + 1) * 512)
                    nc.tensor.matmul(sa2[:, es], qk_sb[0:D, 4 + p, js],
                                     qk_sb[0:D, p, isl], start=True, stop=True)
                for j2 in range(2):
                    jt = 2 * k + j2
                    js = slice(jt * 128, (jt + 1) * 128)
                    es = slice(j2 * 512, (j2 + 1) * 512)
                    nc.tensor.matmul(sb2[:, es], qk_sb[64:64 + D, 4 + p, js],
                                     qk_sb[64:64 + D, p, isl], start=True,
                                     stop=True)
                ea2 = ep.tile([128, 1024], bf16, tag="EA")
                eb2 = ep.tile([128, 1024], bf16, tag="EB")
                with nc.allow_low_precision(reason="schraudolph exp"):
                    nc.scalar.activation(ea2[:], sa2[:], Exp, scale=SCALE)
                    nc.vector.tensor_scalar(
                        out=eb2[:].bitcast(i16), in0=sb2[:],
                        scalar1=A_SCH, scalar2=B_SCH, op0=Mult, op1=Add)
                if pend is not None:
                    emit_ctx(k - 1, *pend)
                pend = (ea2, eb2)
            emit_ctx(7, *pend)
            # den halves -> SBUF (ACT), DMA swaps them onto the ctx lanes;
            # reciprocal later at base partition 0 (custom ucode misbehaves
            # at base partition 64)
            db = dv.tile([128, 512], f32, tag="db", name=f"db{bi}")
            rs = dv.tile([128, 512], f32, tag="rs", name=f"rs{bi}")
            nc.scalar.copy(db[64:128, :], ps_ca[64:128, :])
            nc.scalar.copy(db[0:64, :], ps_cb[0:64, :])
            nc.sync.dma_start(rs[0:64, :], db[64:128, :])
            nc.sync.dma_start(rs[64:128, :], db[0:64, :])
            state[bi] = (ps_ca, ps_cb, rs)

        def emit_div_mults(bi):
            ic, p = blocks[bi]
            isl = slice(ic * 512, (ic + 1) * 512)
            ps_ca, ps_cb, rs = state.pop(bi)
            with tc.high_priority(offset=-60):
                rec = dv.tile([128, 512], f32, tag="rec", name=f"rec{bi}")
                nc.vector.reciprocal_approx_fast(rec[:], rs[:])
                with nc.allow_low_precision(reason="bf16 ctx"):
                    nc.vector.tensor_tensor(
                        out=ctx_sb[0:64, p, isl], in0=ps_ca[0:64, :],
                        in1=rec[0:64, :], op=Mult)
                    nc.vector.tensor_tensor(
                        out=ctx_sb[64:128, p, isl], in0=ps_cb[64:128, :],
                        in1=rec[64:128, :], op=Mult)

        def emit_stage3(ic):
            for tt in range(4):
                tb = ic * 4 + tt
                tsl = slice(tb * 128, (tb + 1) * 128)
                for fc in range(2):
                    fs = slice(fc * 512, (fc + 1) * 512)
                    ps = psC.tile([128, 512], f32, tag="C",
                                  name=f"o{tb}_{fc}")
                    for pb in range(NPAIR):
                        nc.tensor.matmul(
                            ps[:], ctx_sb[:, pb, tsl], wout[pb][:, fs],
                            start=(pb == 0), stop=(pb == NPAIR - 1))
                    o_t = otp.tile([128, 512], bf16, tag="o",
                                   name=f"ot{tb}_{fc}")
                    with nc.allow_low_precision(reason="bf16 out"):
                        if ic == 3 and fc == 1:
                            nc.vector.tensor_copy(o_t[:], ps[:])
                        else:
                            nc.scalar.copy(o_t[:], ps[:])
                    nc.sync.dma_start(out_p[tsl, fs], o_t[:])

        for bi in range(len(blocks) + 2):
            if bi < len(blocks):
                emit_jt_loop(bi)
            if 0 <= bi - 1 < len(blocks):
                emit_div_mults(bi - 1)
            if bi == len(blocks):
                # heaters: keep PE busy (HAM warm) through the final
                # division chain so the last out-projection runs at 2.4GHz
                for hh in range(12):
                    hps = psS.tile([128, 1024], f32, tag="S",
                                   name=f"heat{hh}")
                    nc.tensor.matmul(hps[:, 0:512], qk_sb[:, 0, 0:128],
                                     qk_sb[:, 1, 0:512], start=True,
                                     stop=True)
            if bi - 2 >= 0 and blocks[bi - 2][1] == NPAIR - 1:
                emit_stage3(blocks[bi - 2][0])

    per.release()


_CACHE = {}


def _get_nc(debug=False):
    key = "nc_dbg" if debug else "nc"
    if key not in _CACHE:
        nc = bacc.Bacc("TRN2", target_bir_lowering=False, debug=False)
        xT = nc.dram_tensor("xT", [DM, TOK], bf16, kind="ExternalInput").ap()
        wqkT = nc.dram_tensor("wqkT", [DM, 2 * DL], bf16, kind="ExternalInput").ap()
        wvT = nc.dram_tensor("wvT", [DM, DL], bf16, kind="ExternalInput").ap()
        woutT = nc.dram_tensor("woutT", [DL, DM], bf16, kind="ExternalInput").ap()
        out_p = nc.dram_tensor("out_p", [TOK, DM], bf16, kind="ExternalOutput").ap()
        dbg = None
        if debug:
            dbg = {
                "qk": nc.dram_tensor("qk", [128, 8 * TOK], bf16, kind="ExternalOutput").ap(),
                "v": nc.dram_tensor("v", [128, 16 * NPAIR * 2 * 128], bf16, kind="ExternalOutput").ap(),
                "ctx": nc.dram_tensor("ctx", [128, NPAIR * TOK], bf16, kind="ExternalOutput").ap(),
                "ca0": nc.dram_tensor("ca0", [128, 512], f32, kind="ExternalOutput").ap(),
                "cb0": nc.dram_tensor("cb0", [128, 512], f32, kind="ExternalOutput").ap(),
                "rs0": nc.dram_tensor("rs0", [128, 512], f32, kind="ExternalOutput").ap(),
            }
        with tile.TileContext(nc) as tc:
            _build(tc, xT, wqkT, wvT, woutT, out_p, dbg=dbg)
        nc.compile()
        _CACHE[key] = nc
    return _CACHE[key]


def make_in_maps(x, w_qkv, w_out):
    bf = ml_dtypes.bfloat16
    in_maps = []
    for c in range(N_CORES):
        b, g = c // 2, c % 2
        gs = slice(g * DL, (g + 1) * DL)
        wq = w_qkv[0 * DM + g * DL:0 * DM + (g + 1) * DL]
        wk = w_qkv[1 * DM + g * DL:1 * DM + (g + 1) * DL]
        wv = w_qkv[2 * DM + g * DL:2 * DM + (g + 1) * DL]
        in_maps.append({
            "xT": np.ascontiguousarray(x[b].T).astype(bf),
            "wqkT": np.ascontiguousarray(np.concatenate([wq, wk], 0).T).astype(bf),
            "wvT": np.ascontiguousarray(wv.T).astype(bf),
            "woutT": np.ascontiguousarray(w_out[:, gs].T).astype(bf),
        })
    return in_maps


def kernel(x, w_qkv, w_out, b_out, _trace=False):
    x = np.asarray(x, dtype=np.float32)
    w_qkv = np.asarray(w_qkv, dtype=np.float32)
    w_out = np.asarray(w_out, dtype=np.float32)
    b_out = np.asarray(b_out, dtype=np.float32)

    nc = _get_nc()
    in_maps = make_in_maps(x, w_qkv, w_out)
    res = bass_utils.run_bass_kernel_spmd(
        nc, in_maps, core_ids=list(range(N_CORES)), trace=_trace)
    out = np.empty((4, TOK, DM), dtype=np.float32)
    for b in range(4):
        out[b] = (np.asarray(res.results[2 * b]["out_p"], np.float32)
                  + np.asarray(res.results[2 * b + 1]["out_p"], np.float32))
    out += b_out
    if _trace:
        kernel.last_results = res
    return out
